# revision 9
# baseline (speedup 1.0000x reference)
"""DescriptorLoss kernel for Trainium2 (8 NeuronCores, SPMD data-parallel).

Math:
    d[b,ij,kl] = sum_c desc0[b,c,ij] * desc1[b,c,kl]
    loss = mean(where(mask, 250*relu(1 - d), relu(d - 0.2)))

Per core (shard = (batch, i-slab) -> 1024 ij rows x 4096 kl cols), the PE
computes d' = 5*d via bf16 matmuls into PSUM fp32, in [128 x 2048] chunks
(16 per core). In d' units the hinges sit at 1 and 5:
    5*loss_elem = relu(d'-1)        if m == 0
                  250*relu(5-d')    if m == 1

Chunks are split across two engines:

DVE chunks -- ONE fused custom-DVE pass per chunk (registered at import
into dve_ops.OPS; the per-NEFF DVE table is generated at compile time).
With t = Src1 = (m ? 8192 : 1) (fp8 e5m2, exact) and Src0 = d' (PSUM):
    body = relu(d' - t) + relu((t - d' - 8187) * 250),  accum = sum
  m=0: relu(d'-1) + relu((-8186-d')*250) = relu(d'-1)
  m=1: relu(d'-8192) + relu((5-d')*250)  = 250*relu(5-d')
so accum_out = chunk's exact 5*loss sum. One PSUM read per element.

ACT chunks -- PE injects the mask into PSUM (psum += (-8192*I).T @ m),
giving dM = d' - 8192*m; ACT then runs two relu passes (free affine,
accum_out):
    acc1 = sum relu(dM - 1)      = sum_{m=0} relu(d'-1)
    acc2 = sum relu(-dM - 8187)  = sum_{m=1} relu(5-d')
    chunk sum = acc1 + 250*acc2
"""

import numpy as np
import ml_dtypes
from operator import add

import concourse.bacc as bacc
import concourse.mybir as mybir
import concourse.tile as tile
import concourse.dve_ops as dve_ops_mod
from concourse.dve_spec import Spec, Src0, Src1, C0, C1, relu, lower
from concourse.dve_uop import DveOpSpec
from concourse.bass_utils import run_bass_kernel_spmd

B, D, H, W = 2, 128, 64, 64
N_CORES = 8
IJ = H * W                # 4096
ROWS_PER_CORE = IJ // 4   # 1024
G = ROWS_PER_CORE // 128  # 8 row groups of 128
CH = 2048                 # chunk columns
KT = IJ // CH             # 2 chunks per group
N_CHUNKS = G * KT         # 16
MOFF = 8192.0             # mask offset (exact in fp8 e5m2 / bf16)
LAM = 250.0

# chunks handled by the Scalar (ACT) engine; rest go to the fused DVE op
ACT_CHUNKS = frozenset((2, 5, 8, 11, 14))

_cached = {}

_OP_NAME = "HINGE_PAIR_MASKED_ANT"


def _hinge_ref(in0, in1, s0, s1, imm2):
    x = in0.astype(np.float32)
    t = in1.astype(np.float32)
    out = np.maximum(x - t, 0) + np.maximum((t - x - s0) * s1, 0)
    return out, out.reshape(out.shape[0], -1).sum(axis=-1, keepdims=True).astype(
        np.float32
    )


def _register_dve_op():
    """Register the fused two-hinge op in dve_ops.OPS (documented extension
    point; the uop table is emitted per-NEFF at compile time)."""
    for op in dve_ops_mod.OPS:
        if op.name == _OP_NAME:
            return op
    spec = Spec(
        body=relu(Src0 - Src1) + relu((Src1 - Src0 - C0) * C1),
        accum=add,
        reference=_hinge_ref,
    )
    opcode = dve_ops_mod._CUSTOM_DVE_ROW_BASE + len(dve_ops_mod.OPS)
    shas = {}
    for ver in ("v3", "v4"):
        shas[ver] = DveOpSpec(
            name=_OP_NAME, opcode=opcode, uops=lower(spec, ver=ver), rd1_en=True
        ).sha(ver)
    op = dve_ops_mod.DveOp(_OP_NAME, spec, subdim=False, uops_sha=shas)
    dve_ops_mod.OPS.append(op)
    dve_ops_mod._SUB_OPCODE_FOR_NAME[_OP_NAME] = opcode
    dve_ops_mod.CUSTOM_DVE_SPECS[_OP_NAME] = spec
    return op


_HINGE_OP = _register_dve_op()


def _build_program():
    nc = bacc.Bacc("TRN2")
    f32 = mybir.dt.float32
    bf16 = mybir.dt.bfloat16
    f8 = mybir.dt.float8e5
    Act = mybir.ActivationFunctionType

    aw = nc.declare_dram_parameter("aw", [D, ROWS_PER_CORE], bf16, isOutput=False)
    bm = nc.declare_dram_parameter("bm", [D, IJ], bf16, isOutput=False)
    mv = nc.declare_dram_parameter("mv", [4, 128, 4 * CH], f8, isOutput=False)
    idn = nc.declare_dram_parameter("idn", [D, D], bf16, isOutput=False)
    accs_out = nc.declare_dram_parameter("accs", [128, 2 * N_CHUNKS], f32, isOutput=True)

    with tile.TileContext(nc) as tc:
        with (
            tc.tile_pool(name="desc", bufs=1) as desc_pool,
            tc.tile_pool(name="mask", bufs=4) as mask_pool,
            tc.tile_pool(name="scr", bufs=3) as scr_pool,
            tc.tile_pool(name="accs", bufs=1) as acc_pool,
            tc.tile_pool(name="psd", bufs=2, space="PSUM") as psum_pool,
        ):
            # all 16 mask tiles prefetched via 4 x 1MB DMAs (4 chunks each)
            mgrp = []
            for gq in range(4):
                mg = mask_pool.tile([128, 4 * CH], f8, tag="m", name=f"mg{gq}")
                mgrp.append(mg)
            a_t = desc_pool.tile([D, ROWS_PER_CORE], bf16, tag="a")
            b_lo = desc_pool.tile([D, CH], bf16, tag="blo")
            b_hi = desc_pool.tile([D, CH], bf16, tag="bhi")
            id_t = desc_pool.tile([D, D], bf16, tag="idn")
            warm = desc_pool.tile([128, 8], bf16, tag="warm")
            warm2 = desc_pool.tile([128, 8], bf16, tag="warm2")
            bias_a = desc_pool.tile([128, 1], mybir.dt.float32, tag="ba")
            bias_b = desc_pool.tile([128, 1], mybir.dt.float32, tag="bb")
            nc.sync.dma_start(id_t[:], idn[:])
            nc.sync.dma_start(a_t[:], aw[:])
            nc.sync.dma_start(b_lo[:], bm[:, :CH])
            nc.sync.dma_start(mgrp[0][:], mv[0])
            nc.sync.dma_start(b_hi[:], bm[:, CH:])
            nc.sync.dma_start(mgrp[1][:], mv[1])
            nc.sync.dma_start(mgrp[2][:], mv[2])
            nc.sync.dma_start(mgrp[3][:], mv[3])
            nc.gpsimd.memset(bias_a[:], -1.0)
            nc.gpsimd.memset(bias_b[:], -(MOFF - 5.0))
            # prime the ACT relu table (2.7us one-time) under the input DMAs
            nc.gpsimd.memset(warm[:], 0.0)
            nc.scalar.activation(warm2[:], warm[:], Act.Relu, bias=bias_a[:], scale=1.0)

            acc_t = acc_pool.tile([128, 2 * N_CHUNKS], f32, tag="accs")

            # HAM warm-up: keep the PE busy during the initial DMAs so the
            # clock gate opens (K=8/8) before the real matmuls begin.
            wps = psum_pool.tile([128, CH], f32, tag="d")
            for w in range(48):
                nc.tensor.matmul(
                    wps[:, (w % 4) * 128:(w % 4) * 128 + 128],
                    id_t[:], id_t[:], start=True, stop=True,
                )

            for cid in range(N_CHUNKS):
                g, h = divmod(cid, KT)
                rs = slice(g * 128, (g + 1) * 128)
                on_act = cid in ACT_CHUNKS

                mm_t = mgrp[cid // 4][:, (cid % 4) * CH:(cid % 4 + 1) * CH]
                b_t = b_hi if h else b_lo
                psum_d = psum_pool.tile([128, CH], f32, tag="d")
                n_pass = 3 if on_act else 2
                for p in range(n_pass):
                    for q in range(CH // 512):
                        hs = slice(q * 512, (q + 1) * 512)
                        nc.tensor.matmul(
                            psum_d[:, hs], a_t[:, rs], b_t[:, hs],
                            start=True, stop=(not on_act) and p == n_pass - 1,
                        )
                if on_act:
                    for q in range(CH // 512):
                        hs = slice(q * 512, (q + 1) * 512)
                        nc.tensor.matmul(
                            psum_d[:, hs], id_t[:], mm_t[:, hs],
                            start=False, stop=True,
                        )
                    scr1 = scr_pool.tile([128, CH], bf16, tag="scr")
                    scr2 = scr_pool.tile([128, CH], bf16, tag="scr")
                    nc.scalar.activation(
                        scr1[:], psum_d[:], Act.Relu,
                        bias=bias_a[:], scale=1.0,
                        accum_out=acc_t[:, cid:cid + 1],
                    )
                    nc.scalar.activation(
                        scr2[:], psum_d[:], Act.Relu,
                        bias=bias_b[:], scale=-1.0,
                        accum_out=acc_t[:, N_CHUNKS + cid:N_CHUNKS + cid + 1],
                    )
                else:
                    scr = scr_pool.tile([128, CH], bf16, tag="scr")
                    nc.vector._custom_dve(
                        _HINGE_OP,
                        out=scr[:], in0=psum_d[:], in1=mm_t[:],
                        s0=MOFF - 5.0, s1=LAM,
                        accum_out=acc_t[:, cid:cid + 1],
                    )

            nc.sync.dma_start(accs_out[:], acc_t[:])

    nc.finalize()
    return nc


def _prep_inputs(descriptors_0, descriptors_1, similarity_mask):
    d0 = np.asarray(descriptors_0, dtype=np.float32)
    d1 = np.asarray(descriptors_1, dtype=np.float32)
    mkv = np.asarray(similarity_mask)
    idn = (-MOFF * np.eye(D, dtype=np.float32)).astype(ml_dtypes.bfloat16)
    in_maps = []
    for c in range(N_CORES):
        b = c >> 2
        isl = (c & 3) * 16
        aw = (d0[b].reshape(D, IJ)[:, isl * W:(isl + 16) * W] * np.float32(5.0)).astype(
            ml_dtypes.bfloat16
        )
        bmv = d1[b].reshape(D, IJ).astype(ml_dtypes.bfloat16)
        m = mkv[b, isl:isl + 16].reshape(ROWS_PER_CORE, IJ)
        # chunk tiles [16, 128, CH], then grouped [4, 128, 4*CH]
        mq = m.reshape(G, 128, KT, CH).transpose(0, 2, 1, 3).reshape(N_CHUNKS, 128, CH)
        mvc = np.empty((N_CHUNKS, 128, CH), dtype=ml_dtypes.float8_e5m2)
        for cid in range(N_CHUNKS):
            if cid in ACT_CHUNKS:
                mvc[cid] = mq[cid].astype(ml_dtypes.float8_e5m2)
            else:
                mvc[cid] = np.where(
                    mq[cid], np.float32(MOFF), np.float32(1.0)
                ).astype(ml_dtypes.float8_e5m2)
        mvv = np.ascontiguousarray(
            mvc.reshape(4, 4, 128, CH).transpose(0, 2, 1, 3).reshape(4, 128, 4 * CH)
        )
        in_maps.append(
            {
                "aw": np.ascontiguousarray(aw),
                "bm": np.ascontiguousarray(bmv),
                "mv": mvv,
                "idn": np.ascontiguousarray(idn),
            }
        )
    return in_maps


def _run(in_maps, **kwargs):
    if "nc" not in _cached:
        _cached["nc"] = _build_program()
    return run_bass_kernel_spmd(_cached["nc"], in_maps, list(range(N_CORES)), **kwargs)


def _combine(results):
    total = 0.0
    for r in results:
        accs = r["accs"].astype(np.float64)
        for cid in range(N_CHUNKS):
            if cid in ACT_CHUNKS:
                total += accs[:, cid].sum() + LAM * accs[:, N_CHUNKS + cid].sum()
            else:
                total += accs[:, cid].sum()
    return np.float32(total / 5.0 / float(B * IJ * IJ))


def kernel(descriptors_0, descriptors_1, similarity_mask):
    in_maps = _prep_inputs(descriptors_0, descriptors_1, similarity_mask)
    res = _run(in_maps)
    return _combine(res.results)


# revision 11
# speedup vs baseline: 1.0665x; 1.0665x over previous
"""DescriptorLoss kernel for Trainium2 (8 NeuronCores, SPMD data-parallel).

Math:
    d[b,ij,kl] = sum_c desc0[b,c,ij] * desc1[b,c,kl]
    loss = mean(where(mask, 250*relu(1 - d), relu(d - 0.2)))

Per core (shard = (batch, i-slab) -> 1024 ij rows x 4096 kl cols), the PE
computes d' = 5*d via bf16 matmuls into PSUM fp32, in [128 x 2048] chunks
(16 per core). In d' units the hinges sit at 1 and 5:
    5*loss_elem = relu(d'-1)        if m == 0
                  250*relu(5-d')    if m == 1

Chunks are split across two engines:

DVE chunks -- ONE fused custom-DVE pass per chunk (registered at import
into dve_ops.OPS; the per-NEFF DVE table is generated at compile time).
With t = Src1 = (m ? 8192 : 1) (fp8 e5m2, exact) and Src0 = d' (PSUM):
    body = relu(d' - t) + relu((t - d' - 8187) * 250),  accum = sum
  m=0: relu(d'-1) + relu((-8186-d')*250) = relu(d'-1)
  m=1: relu(d'-8192) + relu((5-d')*250)  = 250*relu(5-d')
so accum_out = chunk's exact 5*loss sum. One PSUM read per element.

ACT chunks -- PE injects the mask into PSUM (psum += (-8192*I).T @ m),
giving dM = d' - 8192*m; ACT then runs two relu passes (free affine,
accum_out):
    acc1 = sum relu(dM - 1)      = sum_{m=0} relu(d'-1)
    acc2 = sum relu(-dM - 8187)  = sum_{m=1} relu(5-d')
    chunk sum = acc1 + 250*acc2
"""

import numpy as np
import ml_dtypes
from operator import add

import concourse.bacc as bacc
import concourse.mybir as mybir
import concourse.tile as tile
import concourse.dve_ops as dve_ops_mod
from concourse.dve_spec import Spec, Src0, Src1, C0, C1, relu, lower
from concourse.dve_uop import DveOpSpec
from concourse.bass_utils import run_bass_kernel_spmd

B, D, H, W = 2, 128, 64, 64
N_CORES = 8
IJ = H * W                # 4096
ROWS_PER_CORE = IJ // 4   # 1024
G = ROWS_PER_CORE // 128  # 8 row groups of 128
CH = 2048                 # chunk columns
KT = IJ // CH             # 2 chunks per group
N_CHUNKS = G * KT         # 16
MOFF = 8192.0             # mask offset (exact in fp8 e5m2 / bf16)
LAM = 250.0

# chunks handled by the Scalar (ACT) engine; rest go to the fused DVE op
ACT_CHUNKS = frozenset((2, 5, 8, 11, 14))

_cached = {}

_OP_NAME = "HINGE_PAIR_MASKED_ANT"


def _hinge_ref(in0, in1, s0, s1, imm2):
    x = in0.astype(np.float32)
    t = in1.astype(np.float32)
    out = np.maximum(x - t, 0) + np.maximum((t - x - s0) * s1, 0)
    return out, out.reshape(out.shape[0], -1).sum(axis=-1, keepdims=True).astype(
        np.float32
    )


def _register_dve_op():
    """Register the fused two-hinge op in dve_ops.OPS (documented extension
    point; the uop table is emitted per-NEFF at compile time)."""
    for op in dve_ops_mod.OPS:
        if op.name == _OP_NAME:
            return op
    spec = Spec(
        body=relu(Src0 - Src1) + relu((Src1 - Src0 - C0) * C1),
        accum=add,
        reference=_hinge_ref,
    )
    opcode = dve_ops_mod._CUSTOM_DVE_ROW_BASE + len(dve_ops_mod.OPS)
    shas = {}
    for ver in ("v3", "v4"):
        shas[ver] = DveOpSpec(
            name=_OP_NAME, opcode=opcode, uops=lower(spec, ver=ver), rd1_en=True
        ).sha(ver)
    op = dve_ops_mod.DveOp(_OP_NAME, spec, subdim=False, uops_sha=shas)
    dve_ops_mod.OPS.append(op)
    dve_ops_mod._SUB_OPCODE_FOR_NAME[_OP_NAME] = opcode
    dve_ops_mod.CUSTOM_DVE_SPECS[_OP_NAME] = spec
    return op


_HINGE_OP = _register_dve_op()


def _build_program():
    nc = bacc.Bacc("TRN2")
    f32 = mybir.dt.float32
    bf16 = mybir.dt.bfloat16
    f8 = mybir.dt.float8e5
    Act = mybir.ActivationFunctionType

    aw = nc.declare_dram_parameter("aw", [D, ROWS_PER_CORE], bf16, isOutput=False)
    bm = nc.declare_dram_parameter("bm", [D, IJ], bf16, isOutput=False)
    mv = nc.declare_dram_parameter("mv", [4, 128, 4 * CH], f8, isOutput=False)
    idn = nc.declare_dram_parameter("idn", [D, D], bf16, isOutput=False)
    accs_out = nc.declare_dram_parameter("accs", [128, 3 * N_CHUNKS], f32, isOutput=True)

    with tile.TileContext(nc) as tc:
        with (
            tc.tile_pool(name="desc", bufs=1) as desc_pool,
            tc.tile_pool(name="mask", bufs=4) as mask_pool,
            tc.tile_pool(name="scr", bufs=3) as scr_pool,
            tc.tile_pool(name="accs", bufs=1) as acc_pool,
            tc.tile_pool(name="psd", bufs=2, space="PSUM") as psum_pool,
        ):
            # all 16 mask tiles prefetched via 4 x 1MB DMAs (4 chunks each)
            mgrp = []
            for gq in range(4):
                mg = mask_pool.tile([128, 4 * CH], f8, tag="m", name=f"mg{gq}")
                mgrp.append(mg)
            a_t = desc_pool.tile([D, ROWS_PER_CORE], bf16, tag="a")
            b_lo = desc_pool.tile([D, CH], bf16, tag="blo")
            b_hi = desc_pool.tile([D, CH], bf16, tag="bhi")
            id_t = desc_pool.tile([D, D], bf16, tag="idn")
            warm = desc_pool.tile([128, 8], bf16, tag="warm")
            warm2 = desc_pool.tile([128, 8], bf16, tag="warm2")
            bias_a = desc_pool.tile([128, 1], mybir.dt.float32, tag="ba")
            bias_b = desc_pool.tile([128, 1], mybir.dt.float32, tag="bb")
            nc.sync.dma_start(id_t[:], idn[:])
            nc.sync.dma_start(a_t[:], aw[:])
            nc.sync.dma_start(b_lo[:], bm[:, :CH])
            nc.sync.dma_start(mgrp[0][:], mv[0])
            nc.sync.dma_start(b_hi[:], bm[:, CH:])
            nc.sync.dma_start(mgrp[1][:], mv[1])
            nc.sync.dma_start(mgrp[2][:], mv[2])
            nc.sync.dma_start(mgrp[3][:], mv[3])
            nc.gpsimd.memset(bias_a[:], -1.0)
            nc.gpsimd.memset(bias_b[:], -(MOFF - 5.0))
            # prime the ACT relu table (2.7us one-time) under the input DMAs
            nc.gpsimd.memset(warm[:], 0.0)
            nc.scalar.activation(warm2[:], warm[:], Act.Relu, bias=bias_a[:], scale=1.0)

            accD_t = acc_pool.tile([128, N_CHUNKS], f32, tag="accsD")
            accA_t = acc_pool.tile([128, 2 * N_CHUNKS], f32, tag="accsA")

            # HAM warm-up: keep the PE busy during the initial DMAs so the
            # clock gate opens (K=8/8) before the real matmuls begin. Junk
            # operand tile (never DMA'd) so this starts immediately.
            junk = desc_pool.tile([128, 128], bf16, tag="junk")
            nc.gpsimd.memset(junk[:], 0.0)
            wps = psum_pool.tile([128, CH], f32, tag="d")
            for w in range(44):
                nc.tensor.matmul(
                    wps[:, (w % 4) * 128:(w % 4) * 128 + 128],
                    junk[:], junk[:], start=True, stop=True,
                )

            for cid in range(N_CHUNKS):
                g, h = divmod(cid, KT)
                rs = slice(g * 128, (g + 1) * 128)
                on_act = cid in ACT_CHUNKS

                mm_t = mgrp[cid // 4][:, (cid % 4) * CH:(cid % 4 + 1) * CH]
                b_t = b_hi if h else b_lo
                psum_d = psum_pool.tile([128, CH], f32, tag="d")
                for q in range(CH // 512):
                    hs = slice(q * 512, (q + 1) * 512)
                    nc.tensor.matmul(
                        psum_d[:, hs], a_t[:, rs], b_t[:, hs],
                        start=True, stop=not on_act,
                    )
                if on_act:
                    for q in range(CH // 512):
                        hs = slice(q * 512, (q + 1) * 512)
                        nc.tensor.matmul(
                            psum_d[:, hs], id_t[:], mm_t[:, hs],
                            start=False, stop=True,
                        )
                    scr1 = scr_pool.tile([128, CH], bf16, tag="scr")
                    scr2 = scr_pool.tile([128, CH], bf16, tag="scr")
                    nc.scalar.activation(
                        scr1[:], psum_d[:], Act.Relu,
                        bias=bias_a[:], scale=1.0,
                        accum_out=accA_t[:, cid:cid + 1],
                    )
                    nc.scalar.activation(
                        scr2[:], psum_d[:], Act.Relu,
                        bias=bias_b[:], scale=-1.0,
                        accum_out=accA_t[:, N_CHUNKS + cid:N_CHUNKS + cid + 1],
                    )
                else:
                    scr = scr_pool.tile([128, CH], bf16, tag="scr")
                    nc.vector._custom_dve(
                        _HINGE_OP,
                        out=scr[:], in0=psum_d[:], in1=mm_t[:],
                        s0=MOFF - 5.0, s1=LAM,
                        accum_out=accD_t[:, cid:cid + 1],
                    )

            nc.sync.dma_start(accs_out[:, :N_CHUNKS], accD_t[:])
            nc.sync.dma_start(accs_out[:, N_CHUNKS:], accA_t[:])

    nc.finalize()
    return nc


def _prep_inputs(descriptors_0, descriptors_1, similarity_mask):
    d0 = np.asarray(descriptors_0, dtype=np.float32)
    d1 = np.asarray(descriptors_1, dtype=np.float32)
    mkv = np.asarray(similarity_mask)
    idn = (-MOFF * np.eye(D, dtype=np.float32)).astype(ml_dtypes.bfloat16)
    in_maps = []
    for c in range(N_CORES):
        b = c >> 2
        isl = (c & 3) * 16
        aw = (d0[b].reshape(D, IJ)[:, isl * W:(isl + 16) * W] * np.float32(5.0)).astype(
            ml_dtypes.bfloat16
        )
        bmv = d1[b].reshape(D, IJ).astype(ml_dtypes.bfloat16)
        m = mkv[b, isl:isl + 16].reshape(ROWS_PER_CORE, IJ)
        # chunk tiles [16, 128, CH], then grouped [4, 128, 4*CH]
        mq = m.reshape(G, 128, KT, CH).transpose(0, 2, 1, 3).reshape(N_CHUNKS, 128, CH)
        mvc = np.empty((N_CHUNKS, 128, CH), dtype=ml_dtypes.float8_e5m2)
        for cid in range(N_CHUNKS):
            if cid in ACT_CHUNKS:
                mvc[cid] = mq[cid].astype(ml_dtypes.float8_e5m2)
            else:
                mvc[cid] = np.where(
                    mq[cid], np.float32(MOFF), np.float32(1.0)
                ).astype(ml_dtypes.float8_e5m2)
        mvv = np.ascontiguousarray(
            mvc.reshape(4, 4, 128, CH).transpose(0, 2, 1, 3).reshape(4, 128, 4 * CH)
        )
        in_maps.append(
            {
                "aw": np.ascontiguousarray(aw),
                "bm": np.ascontiguousarray(bmv),
                "mv": mvv,
                "idn": np.ascontiguousarray(idn),
            }
        )
    return in_maps


def _run(in_maps, **kwargs):
    if "nc" not in _cached:
        _cached["nc"] = _build_program()
    return run_bass_kernel_spmd(_cached["nc"], in_maps, list(range(N_CORES)), **kwargs)


def _combine(results):
    total = 0.0
    for r in results:
        accs = r["accs"].astype(np.float64)
        accD = accs[:, :N_CHUNKS]
        accA = accs[:, N_CHUNKS:]
        for cid in range(N_CHUNKS):
            if cid in ACT_CHUNKS:
                total += accA[:, cid].sum() + LAM * accA[:, N_CHUNKS + cid].sum()
            else:
                total += accD[:, cid].sum()
    return np.float32(total / 5.0 / float(B * IJ * IJ))


def kernel(descriptors_0, descriptors_1, similarity_mask):
    in_maps = _prep_inputs(descriptors_0, descriptors_1, similarity_mask)
    res = _run(in_maps)
    return _combine(res.results)


# revision 14
# speedup vs baseline: 1.4539x; 1.3633x over previous
"""DescriptorLoss kernel for Trainium2 (8 NeuronCores, SPMD data-parallel).

Math:
    d[b,ij,kl] = sum_c desc0[b,c,ij] * desc1[b,c,kl]
    loss = mean(where(mask, 250*relu(1 - d), relu(d - 0.2)))

Per core (shard = (batch, i-slab) -> 1024 ij rows x 4096 kl cols), the PE
computes d' = 5*d via bf16 matmuls into PSUM fp32, in [128 x 2048] chunks
(16 per core). In d' units the hinges sit at 1 and 5:
    5*loss_elem = relu(d'-1)        if m == 0
                  250*relu(5-d')    if m == 1

Chunks are split across two engines:

DVE chunks -- ONE fused custom-DVE pass per chunk (registered at import
into dve_ops.OPS; the per-NEFF DVE table is generated at compile time).
With t = Src1 = (m ? 8192 : 1) (fp8 e5m2, exact) and Src0 = d' (PSUM):
    body = relu(d' - t) + relu((t - d' - 8187) * 250),  accum = sum
  m=0: relu(d'-1) + relu((-8186-d')*250) = relu(d'-1)
  m=1: relu(d'-8192) + relu((5-d')*250)  = 250*relu(5-d')
so accum_out = chunk's exact 5*loss sum. One PSUM read per element.

ACT chunks -- PE injects the mask into PSUM (psum += (-8192*I).T @ m),
giving dM = d' - 8192*m; ACT then runs two relu passes (free affine,
accum_out):
    acc1 = sum relu(dM - 1)      = sum_{m=0} relu(d'-1)
    acc2 = sum relu(-dM - 8187)  = sum_{m=1} relu(5-d')
    chunk sum = acc1 + 250*acc2
"""

import numpy as np
import ml_dtypes
from operator import add

import concourse.bacc as bacc
import concourse.mybir as mybir
import concourse.tile as tile
import concourse.dve_ops as dve_ops_mod
from concourse.dve_spec import Spec, Src0, Src1, C0, C1, relu, lower
from concourse.dve_uop import DveOpSpec
from concourse.bass_utils import run_bass_kernel_spmd

B, D, H, W = 2, 128, 64, 64
N_CORES = 8
IJ = H * W                # 4096
ROWS_PER_CORE = IJ // 4   # 1024
G = ROWS_PER_CORE // 128  # 8 row groups of 128
CH = 1024                 # chunk columns
KT = IJ // CH             # 2 chunks per group
N_CHUNKS = G * KT         # 16
MOFF = 8192.0             # mask offset (exact in fp8 e5m2 / bf16)
LAM = 250.0

# chunks handled by the Scalar (ACT) engine; rest go to the fused DVE op
ACT_CHUNKS = frozenset(range(2, 32, 3))

_cached = {}

_OP_NAME = "HINGE_PAIR_MASKED_ANT"


def _hinge_ref(in0, in1, s0, s1, imm2):
    x = in0.astype(np.float32)
    t = in1.astype(np.float32)
    out = np.maximum(x - t, 0) + np.maximum((t - x - s0) * s1, 0)
    return out, out.reshape(out.shape[0], -1).sum(axis=-1, keepdims=True).astype(
        np.float32
    )


def _register_dve_op():
    """Register the fused two-hinge op in dve_ops.OPS (documented extension
    point; the uop table is emitted per-NEFF at compile time)."""
    for op in dve_ops_mod.OPS:
        if op.name == _OP_NAME:
            return op
    spec = Spec(
        body=relu(Src0 - Src1) + relu((Src1 - Src0 - C0) * C1),
        accum=add,
        reference=_hinge_ref,
    )
    opcode = dve_ops_mod._CUSTOM_DVE_ROW_BASE + len(dve_ops_mod.OPS)
    shas = {}
    for ver in ("v3", "v4"):
        shas[ver] = DveOpSpec(
            name=_OP_NAME, opcode=opcode, uops=lower(spec, ver=ver), rd1_en=True
        ).sha(ver)
    op = dve_ops_mod.DveOp(_OP_NAME, spec, subdim=False, uops_sha=shas)
    dve_ops_mod.OPS.append(op)
    dve_ops_mod._SUB_OPCODE_FOR_NAME[_OP_NAME] = opcode
    dve_ops_mod.CUSTOM_DVE_SPECS[_OP_NAME] = spec
    return op


_HINGE_OP = _register_dve_op()


def _build_program():
    nc = bacc.Bacc("TRN2")
    f32 = mybir.dt.float32
    bf16 = mybir.dt.bfloat16
    f8 = mybir.dt.float8e5
    Act = mybir.ActivationFunctionType

    aw = nc.declare_dram_parameter("aw", [D, ROWS_PER_CORE], bf16, isOutput=False)
    bm = nc.declare_dram_parameter("bm", [D, IJ], bf16, isOutput=False)
    mv = nc.declare_dram_parameter("mv", [4, 128, 8 * CH], f8, isOutput=False)
    idn = nc.declare_dram_parameter("idn", [D, D], bf16, isOutput=False)
    accs_out = nc.declare_dram_parameter("accs", [128, 3 * N_CHUNKS], f32, isOutput=True)

    with tile.TileContext(nc) as tc:
        with (
            tc.tile_pool(name="desc", bufs=1) as desc_pool,
            tc.tile_pool(name="mask", bufs=4) as mask_pool,
            tc.tile_pool(name="scr", bufs=3) as scr_pool,
            tc.tile_pool(name="accs", bufs=1) as acc_pool,
            tc.tile_pool(name="psd", bufs=4, space="PSUM") as psum_pool,
        ):
            # all 16 mask tiles prefetched via 4 x 1MB DMAs (4 chunks each)
            mgrp = []
            for gq in range(4):
                mg = mask_pool.tile([128, 8 * CH], f8, tag="m", name=f"mg{gq}")
                mgrp.append(mg)
            a_t = desc_pool.tile([D, ROWS_PER_CORE], bf16, tag="a")
            b_lo = desc_pool.tile([D, IJ // 2], bf16, tag="blo")
            b_hi = desc_pool.tile([D, IJ // 2], bf16, tag="bhi")
            id_t = desc_pool.tile([D, D], bf16, tag="idn")
            warm = desc_pool.tile([128, 8], bf16, tag="warm")
            warm2 = desc_pool.tile([128, 8], bf16, tag="warm2")
            bias_a = desc_pool.tile([128, 1], mybir.dt.float32, tag="ba")
            bias_b = desc_pool.tile([128, 1], mybir.dt.float32, tag="bb")
            nc.sync.dma_start(id_t[:], idn[:])
            nc.sync.dma_start(a_t[:], aw[:])
            nc.sync.dma_start(b_lo[:], bm[:, :IJ // 2])
            nc.sync.dma_start(mgrp[0][:], mv[0])
            nc.sync.dma_start(b_hi[:], bm[:, IJ // 2:])
            nc.sync.dma_start(mgrp[1][:], mv[1])
            nc.sync.dma_start(mgrp[2][:], mv[2])
            nc.sync.dma_start(mgrp[3][:], mv[3])
            nc.gpsimd.memset(bias_a[:], -1.0)
            nc.gpsimd.memset(bias_b[:], -(MOFF - 5.0))
            # prime the ACT relu table (2.7us one-time) under the input DMAs
            nc.gpsimd.memset(warm[:], 0.0)
            nc.scalar.activation(warm2[:], warm[:], Act.Relu, bias=bias_a[:], scale=1.0)

            accD_t = acc_pool.tile([128, N_CHUNKS], f32, tag="accsD")
            accA_t = acc_pool.tile([128, 2 * N_CHUNKS], f32, tag="accsA")

            # HAM warm-up: keep the PE busy during the initial DMAs so the
            # clock gate opens (K=8/8) before the real matmuls begin. Junk
            # operand tile (never DMA'd) so this starts immediately.
            junk = desc_pool.tile([128, 128], bf16, tag="junk")
            nc.gpsimd.memset(junk[:], 0.0)
            wps = psum_pool.tile([128, CH], f32, tag="d")
            for w in range(20):
                nc.tensor.matmul(
                    wps[:, (w % 4) * 128:(w % 4) * 128 + 128],
                    junk[:], junk[:], start=True, stop=True,
                )

            for cid in range(N_CHUNKS):
                g, h = divmod(cid, KT)
                rs = slice(g * 128, (g + 1) * 128)
                on_act = cid in ACT_CHUNKS

                mm_t = mgrp[cid // 8][:, (cid % 8) * CH:(cid % 8 + 1) * CH]
                b_t = b_lo if h < 2 else b_hi
                boff = (h % 2) * CH
                psum_d = psum_pool.tile([128, CH], f32, tag="d")
                for q in range(CH // 512):
                    hs = slice(q * 512, (q + 1) * 512)
                    bs = slice(boff + q * 512, boff + (q + 1) * 512)
                    nc.tensor.matmul(
                        psum_d[:, hs], a_t[:, rs], b_t[:, bs],
                        start=True, stop=not on_act,
                    )
                if on_act:
                    for q in range(CH // 512):
                        hs = slice(q * 512, (q + 1) * 512)
                        nc.tensor.matmul(
                            psum_d[:, hs], id_t[:], mm_t[:, hs],
                            start=False, stop=True,
                        )
                    scr1 = scr_pool.tile([128, CH], bf16, tag="scr")
                    scr2 = scr_pool.tile([128, CH], bf16, tag="scr")
                    nc.scalar.activation(
                        scr1[:], psum_d[:], Act.Relu,
                        bias=bias_a[:], scale=1.0,
                        accum_out=accA_t[:, cid:cid + 1],
                    )
                    nc.scalar.activation(
                        scr2[:], psum_d[:], Act.Relu,
                        bias=bias_b[:], scale=-1.0,
                        accum_out=accA_t[:, N_CHUNKS + cid:N_CHUNKS + cid + 1],
                    )
                else:
                    scr = scr_pool.tile([128, CH], bf16, tag="scr")
                    nc.vector._custom_dve(
                        _HINGE_OP,
                        out=scr[:], in0=psum_d[:], in1=mm_t[:],
                        s0=MOFF - 5.0, s1=LAM,
                        accum_out=accD_t[:, cid:cid + 1],
                    )

            nc.sync.dma_start(accs_out[:, :N_CHUNKS], accD_t[:])
            nc.sync.dma_start(accs_out[:, N_CHUNKS:], accA_t[:])

    nc.finalize()
    return nc


def _prep_inputs(descriptors_0, descriptors_1, similarity_mask):
    d0 = np.asarray(descriptors_0, dtype=np.float32)
    d1 = np.asarray(descriptors_1, dtype=np.float32)
    mkv = np.asarray(similarity_mask)
    idn = (-MOFF * np.eye(D, dtype=np.float32)).astype(ml_dtypes.bfloat16)
    in_maps = []
    for c in range(N_CORES):
        b = c >> 2
        isl = (c & 3) * 16
        aw = (d0[b].reshape(D, IJ)[:, isl * W:(isl + 16) * W] * np.float32(5.0)).astype(
            ml_dtypes.bfloat16
        )
        bmv = d1[b].reshape(D, IJ).astype(ml_dtypes.bfloat16)
        m = mkv[b, isl:isl + 16].reshape(ROWS_PER_CORE, IJ)
        # chunk tiles [16, 128, CH], then grouped [4, 128, 4*CH]
        mq = m.reshape(G, 128, KT, CH).transpose(0, 2, 1, 3).reshape(N_CHUNKS, 128, CH)
        mvc = np.empty((N_CHUNKS, 128, CH), dtype=ml_dtypes.float8_e5m2)
        for cid in range(N_CHUNKS):
            if cid in ACT_CHUNKS:
                mvc[cid] = mq[cid].astype(ml_dtypes.float8_e5m2)
            else:
                mvc[cid] = np.where(
                    mq[cid], np.float32(MOFF), np.float32(1.0)
                ).astype(ml_dtypes.float8_e5m2)
        mvv = np.ascontiguousarray(
            mvc.reshape(4, 8, 128, CH).transpose(0, 2, 1, 3).reshape(4, 128, 8 * CH)
        )
        in_maps.append(
            {
                "aw": np.ascontiguousarray(aw),
                "bm": np.ascontiguousarray(bmv),
                "mv": mvv,
                "idn": np.ascontiguousarray(idn),
            }
        )
    return in_maps


def _run(in_maps, **kwargs):
    if "nc" not in _cached:
        _cached["nc"] = _build_program()
    return run_bass_kernel_spmd(_cached["nc"], in_maps, list(range(N_CORES)), **kwargs)


def _combine(results):
    total = 0.0
    for r in results:
        accs = r["accs"].astype(np.float64)
        accD = accs[:, :N_CHUNKS]
        accA = accs[:, N_CHUNKS:]
        for cid in range(N_CHUNKS):
            if cid in ACT_CHUNKS:
                total += accA[:, cid].sum() + LAM * accA[:, N_CHUNKS + cid].sum()
            else:
                total += accD[:, cid].sum()
    return np.float32(total / 5.0 / float(B * IJ * IJ))


def kernel(descriptors_0, descriptors_1, similarity_mask):
    in_maps = _prep_inputs(descriptors_0, descriptors_1, similarity_mask)
    res = _run(in_maps)
    return _combine(res.results)
